# revision 21
# baseline (speedup 1.0000x reference)
"""ALBERT attention (B=2, S=2048, D=1024, H=16, K=64) on 8 TRN2 NeuronCores.

Sharding: core c = (b, g) with b = c // 4 (batch), g = c % 4 (head group of 4
heads). Each core computes output[b, :, 4g:4g+4, :] — outputs are disjoint, so
no collectives are needed.

Host-side prep: keys with attention_mask == 0 are compacted away (they
contribute exactly 0), padded to a 128 multiple; only the LAST key tile
contains masked (padding) keys, so only its exp() needs the additive-mask
bias.  ALL inputs are host-packed into the EXACT SBUF tile layout
([128 partitions, free]) so every DMA moves >=4KB-contiguous per partition
(128 big descriptors per transfer instead of thousands of 256B ones).

Per-core pipeline (ScalarE exp is the roofline: 64 ACTs x ~1.11us):
  - DMAs ride the two HW-DGE queues (scalar, sync) for the critical prefix
    (wqk0 / xt parts 0-1 / xf quarter 0 / wv) and the gpsimd SW-DGE queue for
    late-needed data.  With packed layouts each dma_start occupies its engine
    for <1us, so the scalar queue is free for the exp stream.
  - projections, weight-stationary, bf16: QT [2-head 128, S] per pair and
    KT [128, T] per pair; V computed DIRECTLY in [t, hk] layout (xt chunks
    stationary, wv moving) — no PE transpose pass.
  - attention runs per (head-PAIR, f-quarter 512): the two heads' logits
    matmuls contract 64 rows each at partition offsets 0 / 64 so the PE runs
    them CONCURRENTLY into one lt [128, 1024] tile ([A | B]); a single exp
    ACT covers both; per-head contexts Cacc[65, 512] += [1|V]^T @ ET (row 0 =
    softmax denominators). PSUM: lt 2x2 + cacc 2x1 + proj 2 = 8 banks.
  - non-critical projections are chopped into SINGLE-MATMUL ops and INJECTED
    between attention iterations (program order = Tile priority), so the PE
    slack under the exp stream absorbs them instead of stalling it.
  - output ships UNNORMALIZED in bf16: out[h] = [65, S] (row 0 = denom,
    rows 1..65 = C^T). Host divides, transposes, and adds bv (exact since
    probs sum to 1).
"""

import ml_dtypes
import numpy as np

import concourse.bass as bass
import concourse.tile as tile
from concourse import bacc, mybir
from concourse.bass_utils import run_bass_kernel_spmd

F32 = mybir.dt.float32
BF16 = mybir.dt.bfloat16

B, S, D, H, K = 2, 2048, 1024, 16, 64
NH = 4            # heads per core
HK = NH * K       # 256
NCORES = 8
DC = D // 128     # 8 contraction chunks
NEG = -10000.0
FQ = 512          # f-quarter width
NQ = S // FQ      # 4 f-quarters
TP = 256          # xt part width == K-projection chunk width

# schedule knobs
N_WARM_BIG = 8    # 512-col warmup matmuls
N_WARM_SMALL = 8  # 128-col tail warmup matmuls (drain fast when work lands)
INJ = {(0, 0): 8, (0, 1): 4, (0, 2): 4, (0, 3): 2,
       (1, 0): 2, (1, 1): 2, (1, 2): 2, (1, 3): 0}


import os

_BISECT = os.environ.get("KBISECT", "")


def _part_widths(T):
    """xt part widths.  MUST be uniform: non-uniform xt tensor widths
    produce a NEFF the runtime rejects (INVALID_ARGUMENT) even though
    CoreSim validates the graph — bisected 2026-08-09."""
    w0 = 128 if "tp128" in _BISECT else 256
    ws, off = [], 0
    while off < T:
        w = min(w0, T - off)
        ws.append(w)
        off += w
    return ws


def build_nc(t_tiles: int):
    """Per-core Bass graph. t_tiles = number of 128-row key tiles after
    host-side compaction of masked-out keys."""
    T = t_tiles * 128
    pw = []                      # xt parts: (col offset, width)
    off = 0
    for w in _part_widths(T):
        pw.append((off, w))
        off += w
    NP = len(pw)

    nc = bacc.Bacc("TRN2", target_bir_lowering=False, debug=False,
                   num_devices=NCORES)

    # all inputs pre-packed host-side into [128, free] SBUF layout
    xf_d = [nc.dram_tensor(f"xf{j}", [128, DC * FQ], BF16,
                           kind="ExternalInput").ap() for j in range(NQ)]
    xt_d = [nc.dram_tensor(f"xt{i}", [128, DC * w], BF16,
                           kind="ExternalInput").ap()
            for i, (_, w) in enumerate(pw)]
    wqk_d = [nc.dram_tensor(f"wqk{hk}", [128, 2 * DC * 128], BF16,
                            kind="ExternalInput").ap() for hk in range(2)]
    wv_d = nc.dram_tensor("wv", [128, DC * HK], BF16,
                          kind="ExternalInput").ap()
    # bias columns: [bq0 bq1 bk0 bk1] (hk-tile halves of bq / bk)
    bias_d = nc.dram_tensor("bias", [128, 4], F32, kind="ExternalInput").ap()
    # additive key mask for the LAST key tile only
    mask_d = nc.dram_tensor("mask", [128, 1], F32, kind="ExternalInput").ap()
    # unnormalized: per head, row 0 = softmax denominators, rows 1..64 = C^T
    out_d = nc.dram_tensor("out", [NH, K + 1, S], BF16,
                           kind="ExternalOutput").ap()

    with tile.TileContext(nc) as tc:
        with (
            tc.sbuf_pool(name="const", bufs=1) as const_pool,
            tc.sbuf_pool(name="persist", bufs=1) as persist_pool,
            tc.psum_pool(name="proj", bufs=2) as proj_pool,
            tc.psum_pool(name="cacc", bufs=2) as cacc_pool,
            tc.sbuf_pool(name="et", bufs=t_tiles + 2) as et_pool,
            tc.sbuf_pool(name="ct", bufs=4) as ct_pool,
        ):
            bias_sb = const_pool.tile([128, 4], F32)
            mask_sb = const_pool.tile([128, 1], F32)
            warm_sb = const_pool.tile([128, 512], BF16)

            xf_sb = [persist_pool.tile([128, DC * FQ], BF16, name=f"xf{j}")
                     for j in range(NQ)]
            xt_sb = [persist_pool.tile([128, DC * w], BF16, name=f"xt{i}")
                     for i, (_, w) in enumerate(pw)]
            wqk_sb = [persist_pool.tile([128, 2 * DC * 128], BF16,
                                        name=f"wqk{hk}") for hk in range(2)]
            wv_sb = persist_pool.tile([128, DC * HK], BF16, name="wv")

            def wq_v(hk, d):     # [128, 128] stationary chunk
                return wqk_sb[hk][:, 128 * d:128 * (d + 1)]

            def wk_v(hk, d):
                return wqk_sb[hk][:, DC * 128 + 128 * d:
                                  DC * 128 + 128 * (d + 1)]

            def wv_v(d):         # [128, 256] moving chunk
                return wv_sb[:, HK * d:HK * (d + 1)]

            def xt_v(d, c0, w):  # [128, w] view of xt chunk d cols c0:c0+w
                for i, (o, wi) in enumerate(pw):
                    if o <= c0 and c0 + w <= o + wi:
                        s = wi * d + (c0 - o)
                        return xt_sb[i][:, s:s + w]
                raise AssertionError((c0, w))

            def xf_v(d, q):      # [128, 512] view of xf chunk d, quarter q
                return xf_sb[q][:, FQ * d:FQ * (d + 1)]

            qt_sb = [persist_pool.tile([128, S], BF16, name=f"qt{i}")
                     for i in range(2)]
            kt_sb = [persist_pool.tile([128, T], BF16, name=f"kt{i}")
                     for i in range(2)]
            # V with a leading ones column per head: [1|V_h0|1|V_h1|...]
            v_sb = [persist_pool.tile([128, NH * 65], BF16, name=f"v{i}")
                    for i in range(t_tiles)]
            nc.vector.memset(warm_sb[:], 0.0)
            for i in range(t_tiles):
                nc.vector.memset(
                    v_sb[i].rearrange("p (h c) -> p h c", c=65)[:, :, 0:1],
                    1.0)

            # ---------------- input DMAs ----------------
            # The two HW-DGE queues (scalar + sync) carry the critical
            # prefix: everything attention(0,0) touches.  Late-needed data
            # (xf quarters 1-3, pair-1 weights) rides the gpsimd SW-DGE
            # queue GATED behind the Q00 drain so it cannot steal HBM
            # bandwidth from the critical prefix.  Packed layouts make each
            # dma_start issue in <1us of engine time.
            if "v1dma" in _BISECT:
                nc.scalar.dma_start(bias_sb[:], bias_d[:])
                nc.scalar.dma_start(mask_sb[:], mask_d[:])
                nc.scalar.dma_start(wqk_sb[0][:], wqk_d[0][:])
                nc.scalar.dma_start(xt_sb[0][:], xt_d[0][:])
                nc.scalar.dma_start(xt_sb[1][:], xt_d[1][:])
                nc.sync.dma_start(xf_sb[0][:], xf_d[0][:])
                nc.sync.dma_start(wv_sb[:], wv_d[:])
                nc.sync.dma_start(wqk_sb[1][:], wqk_d[1][:])
                nc.sync.dma_start(xf_sb[1][:], xf_d[1][:])
                for i in range(2, NP):
                    nc.gpsimd.dma_start(xt_sb[i][:], xt_d[i][:])
                nc.gpsimd.dma_start(xf_sb[2][:], xf_d[2][:])
                nc.gpsimd.dma_start(xf_sb[3][:], xf_d[3][:])
            else:
                # critical prefix on the two HW-DGE queues; xf0 split in
                # half across both so Q00 unblocks as early as possible
                HF = (4 * FQ) if "nosplit" not in _BISECT else 0
                nc.scalar.dma_start(bias_sb[:], bias_d[:])
                nc.scalar.dma_start(mask_sb[:], mask_d[:])
                nc.scalar.dma_start(wqk_sb[0][:], wqk_d[0][:])
                if HF:
                    nc.scalar.dma_start(xf_sb[0][:, HF:], xf_d[0][:, HF:])
                nc.scalar.dma_start(wv_sb[:], wv_d[:])
                for i in range(NP):
                    if i == 2:
                        nc.sync.dma_start(xf_sb[0][:, 0:HF] if HF
                                          else xf_sb[0][:],
                                          xf_d[0][:, 0:HF] if HF
                                          else xf_d[0][:])
                    nc.sync.dma_start(xt_sb[i][:], xt_d[i][:])
                if NP <= 2:
                    nc.sync.dma_start(xf_sb[0][:, 0:HF] if HF else xf_sb[0][:],
                                      xf_d[0][:, 0:HF] if HF else xf_d[0][:])
                # late-needed inputs ride the gpsimd SW-DGE queue, gated
                # behind the Q00 drain (see schedule section) so they can't
                # steal HBM bandwidth from the critical prefix

            # -------- projection pieces as lists of small ops --------
            def ops_q(hk, q):
                st = {}
                ops = [lambda: st.__setitem__(
                    "ps", proj_pool.tile([128, FQ], F32, tag="pp",
                                         name=f"qp{hk}_{q}"))]
                for d in range(DC):
                    ops.append(lambda d=d: nc.tensor.matmul(
                        st["ps"][:], wq_v(hk, d), xf_v(d, q),
                        start=(d == 0), stop=(d == DC - 1)))
                ops.append(lambda: nc.vector.tensor_scalar_add(
                    qt_sb[hk][:, FQ * q:FQ * (q + 1)], st["ps"][:],
                    bias_sb[:, hk:hk + 1]))
                return ops

            def ops_k(hk, ci):
                c0, w = pw[ci]
                st = {}
                ops = [lambda: st.__setitem__(
                    "ps", proj_pool.tile([128, w], F32, tag="pp",
                                         name=f"kp{hk}_{ci}"))]
                for d in range(DC):
                    ops.append(lambda d=d: nc.tensor.matmul(
                        st["ps"][:], wk_v(hk, d), xt_v(d, c0, w),
                        start=(d == 0), stop=(d == DC - 1)))
                ops.append(lambda: nc.vector.tensor_scalar_add(
                    kt_sb[hk][:, c0:c0 + w], st["ps"][:],
                    bias_sb[:, 2 + hk:3 + hk]))
                return ops

            def ops_v(t):
                """bv is added on the host (exact: probs sum to 1)."""
                st = {}
                ops = [lambda: st.__setitem__(
                    "ps", proj_pool.tile([128, HK], F32, tag="pp",
                                         name=f"vp{t}"))]
                for d in range(DC):
                    ops.append(lambda d=d: nc.tensor.matmul(
                        st["ps"][:], xt_v(d, 128 * t, 128), wv_v(d),
                        start=(d == 0), stop=(d == DC - 1)))
                ops.append(lambda: nc.vector.tensor_copy(
                    v_sb[t].rearrange("p (h c) -> p h c", c=65)[:, :, 1:65],
                    st["ps"].rearrange("p (h c) -> p h c", c=64)[:, :, :]))
                return ops

            # pending background ops: (deadline_slot, op).  deadline_slot is
            # the absolute attention slot (call_index * t_tiles + tile)
            # BEFORE whose logits matmul the op must have executed; the list
            # is kept in non-decreasing deadline order so FIFO execution is
            # both the pacing order and the correctness order.
            bg = []

            def bg_add(deadline, piece):
                assert not bg or bg[-1][0] <= deadline
                bg.extend((deadline, op) for op in piece)

            def bg_pop(n):
                for _ in range(min(n, len(bg))):
                    bg.pop(0)[1]()

            def bg_run_due(slot):
                while bg and bg[0][0] <= slot:
                    bg.pop(0)[1]()

            def bg_flush():
                bg_pop(len(bg))

            def attention(hk, fq, lt_pool, base, inject=3):
                """Both heads of pair hk on f-quarter fq: concurrent
                row-tiled logits -> one exp ACT -> per-head contexts."""
                hA, hB = 2 * hk, 2 * hk + 1
                c0 = FQ * fq
                caccs = [cacc_pool.tile([K + 1, FQ], F32, tag="cacc",
                                        name=f"cacc{hk}_{fq}_{i}")
                         for i in range(2)]
                for t in range(t_tiles):
                    bg_run_due(base + t)
                    lt = lt_pool.tile([128, 2 * FQ], F32, tag="lt",
                                      name=f"lt{hk}_{fq}_{t}")
                    for i, zo in ((0, 0), (1, 64)):
                        nc.tensor.matmul(
                            lt[:, FQ * i:FQ * (i + 1)],
                            kt_sb[hk][zo:zo + 64, 128 * t:128 * (t + 1)],
                            qt_sb[hk][zo:zo + 64, c0:c0 + FQ],
                            start=True, stop=True)
                    et = et_pool.tile([128, 2 * FQ], BF16, tag="et",
                                      name=f"et{hk}_{fq}_{t}")
                    nc.scalar.activation(
                        et[:], lt[:], mybir.ActivationFunctionType.Exp,
                        bias=(mask_sb[:, 0:1] if t == t_tiles - 1 else 0.0),
                        scale=0.125)
                    bg_pop(inject)
                    for i, h in ((0, hA), (1, hB)):
                        nc.tensor.matmul(
                            caccs[i][:],
                            v_sb[t][:, 65 * h:65 * (h + 1)],
                            et[:, FQ * i:FQ * (i + 1)],
                            start=(t == 0), stop=(t == t_tiles - 1),
                            skip_group_check=True)
                for i, h in ((0, hA), (1, hB)):
                    ct = ct_pool.tile([K + 1, FQ], BF16, tag="ct",
                                      name=f"ct{hk}_{fq}_{i}")
                    nc.vector.tensor_copy(ct[:], caccs[i][:])
                    nc.sync.dma_start(out_d[h][:, c0:c0 + FQ], ct[:])

            # ---------------- schedule ----------------
            # PE warmup: keep the HAM clock-gate hot until real work lands;
            # small tail matmuls drain fast once inputs arrive.
            with tc.psum_pool(name="warm", bufs=1) as warm_pool:
                wps = warm_pool.tile([128, 512], F32, tag="wp", name="warm")
                for _ in range(N_WARM_BIG):
                    nc.tensor.matmul(wps[:], warm_sb[:, 0:128], warm_sb[:],
                                     start=True, stop=True)
                for _ in range(N_WARM_SMALL):
                    nc.tensor.matmul(wps[:, 0:128], warm_sb[:, 0:128],
                                     warm_sb[:, 0:128], start=True, stop=True)

            # upfront: what attention(0,0) tiles 0/1 need, ordered to match
            # DMA arrival (xt0, wv, xt1 land before xf0's second half)
            up_parts = min(3 if "up3" in _BISECT else 2, NP)
            pieces = []
            for ci in range(up_parts):
                pieces.append(ops_k(0, ci))
                for t in range(pw[ci][0] // 128,
                               (pw[ci][0] + pw[ci][1]) // 128):
                    pieces.append(ops_v(t))
            pieces.append(ops_q(0, 0))
            for piece in pieces:
                for op in piece:
                    op()

            # gated SW-DGE DMAs: the gpsimd queue only starts pulling HBM
            # bandwidth once the Q00 drain has run (i.e. the critical
            # prefix has landed).  Not under v1dma (those DMAs were issued
            # upfront there).
            if "v1dma" not in _BISECT:
                if "nogate" not in _BISECT:
                    gate_sb = const_pool.tile([128, 1], BF16)
                    nc.gpsimd.tensor_copy(gate_sb[:], qt_sb[0][:, 0:1])
                nc.gpsimd.dma_start(xf_sb[1][:], xf_d[1][:])
                nc.gpsimd.dma_start(xf_sb[2][:], xf_d[2][:])
                nc.gpsimd.dma_start(wqk_sb[1][:], wqk_d[1][:])
                nc.gpsimd.dma_start(xf_sb[3][:], xf_d[3][:])

            # background work with absolute slot deadlines
            TT = t_tiles
            for ci in range(up_parts, NP):
                t0c = pw[ci][0] // 128
                bg_add(t0c, ops_k(0, ci))
                for t in range(t0c, (pw[ci][0] + pw[ci][1]) // 128):
                    bg_add(t, ops_v(t))
            bg_add(1 * TT, ops_q(0, 1))
            bg_add(2 * TT, ops_q(0, 2))
            bg_add(3 * TT, ops_q(0, 3))
            bg_add(4 * TT, ops_q(1, 0))
            for ci in range(NP):
                bg_add(4 * TT + pw[ci][0] // 128, ops_k(1, ci))
            bg_add(5 * TT, ops_q(1, 1))
            bg_add(6 * TT, ops_q(1, 2))
            bg_add(7 * TT, ops_q(1, 3))

            with tc.psum_pool(name="lt", bufs=2) as lt_pool:
                for ci, (hk, fq) in enumerate(
                        [(h, q) for h in range(2) for q in range(NQ)]):
                    attention(hk, fq, lt_pool, base=ci * TT,
                              inject=INJ[(hk, fq)])
                bg_flush()

    nc.compile()
    return nc


_NC_CACHE = {}


def _get_nc(t_tiles: int):
    if t_tiles not in _NC_CACHE:
        _NC_CACHE[t_tiles] = build_nc(t_tiles)
    return _NC_CACHE[t_tiles]


def _pack_cols(a):
    """[D, W] (d*128+p, w) -> [128, DC*W] SBUF layout [p, d*W+w]."""
    Wd = a.shape[1]
    return np.ascontiguousarray(
        a.reshape(DC, 128, Wd).transpose(1, 0, 2).reshape(128, DC * Wd))


def kernel(from_tensor, to_tensor, attention_mask, Wq, bq, Wk, bk, Wv, bv):
    from_tensor = np.asarray(from_tensor, dtype=np.float32)
    to_tensor = np.asarray(to_tensor, dtype=np.float32)
    attention_mask = np.asarray(attention_mask)
    Wq = np.asarray(Wq, dtype=np.float32)
    Wk = np.asarray(Wk, dtype=np.float32)
    Wv = np.asarray(Wv, dtype=np.float32)
    bq = np.asarray(bq, dtype=np.float32)
    bk = np.asarray(bk, dtype=np.float32)
    bv = np.asarray(bv, dtype=np.float32)

    # compact away masked-out keys (they contribute exactly 0 to the
    # context); pad to a 128 multiple and re-mask the padding tail.
    mask_np = attention_mask.astype(np.int32)
    idxs = [np.nonzero(mask_np[b])[0] for b in range(B)]
    t_eff = max(1, max(len(ix) for ix in idxs))
    T_pad = min(S, ((t_eff + 127) // 128) * 128)
    t_tiles = T_pad // 128
    nc = _get_nc(t_tiles)

    pw = []
    off = 0
    for w in _part_widths(T_pad):
        pw.append((off, w))
        off += w

    # per-batch packed activations (shared by the 4 cores of each batch)
    xf_pack = []   # [b][j] -> [128, DC*FQ] bf16
    xt_pack = []   # [b][i] -> [128, DC*w] bf16
    maskadd = np.full((B, T_pad), NEG, dtype=np.float32)
    for b in range(B):
        ix = idxs[b]
        xfT = np.ascontiguousarray(from_tensor[b].T).astype(ml_dtypes.bfloat16)
        xf_pack.append([_pack_cols(xfT[:, FQ * j:FQ * (j + 1)])
                        for j in range(NQ)])
        xt_c = np.zeros((D, T_pad), dtype=np.float32)
        xt_c[:, :len(ix)] = to_tensor[b].T[:, ix]
        xt_b = xt_c.astype(ml_dtypes.bfloat16)
        xt_pack.append([_pack_cols(xt_b[:, o:o + w]) for o, w in pw])
        maskadd[b, :len(ix)] = 0.0

    in_maps = []
    for c in range(NCORES):
        b, g = c // 4, c % 4
        hs = slice(NH * g, NH * (g + 1))
        wq_sl = Wq[:, hs, :].reshape(D, HK)
        wk_sl = Wk[:, hs, :].reshape(D, HK)
        wv_sl = Wv[:, hs, :].reshape(D, HK)
        im = {
            "wv": _pack_cols(wv_sl).astype(ml_dtypes.bfloat16),
            "bias": np.ascontiguousarray(np.stack([
                bq[hs].reshape(HK)[:128], bq[hs].reshape(HK)[128:],
                bk[hs].reshape(HK)[:128], bk[hs].reshape(HK)[128:],
            ], axis=1)),
            "mask": np.ascontiguousarray(
                maskadd[b][(t_tiles - 1) * 128:].reshape(128, 1)),
        }
        for hk in range(2):
            sl = slice(128 * hk, 128 * (hk + 1))
            im[f"wqk{hk}"] = np.concatenate(
                [_pack_cols(wq_sl[:, sl]), _pack_cols(wk_sl[:, sl])],
                axis=1).astype(ml_dtypes.bfloat16)
        for j in range(NQ):
            im[f"xf{j}"] = xf_pack[b][j]
        for i in range(len(pw)):
            im[f"xt{i}"] = xt_pack[b][i]
        in_maps.append(im)

    global _LAST_IN_MAPS, _LAST_T_TILES
    _LAST_IN_MAPS = in_maps
    _LAST_T_TILES = t_tiles
    try:
        res = run_bass_kernel_spmd(nc, in_maps, core_ids=list(range(NCORES)))
    except Exception:
        # the axon terminal occasionally reports the device unrecoverable;
        # a reset + retry clears it
        try:
            import ctypes

            lib = ctypes.CDLL("/opt/axon/libaxon_pjrt.so")
            lib.axon_reset.restype = ctypes.c_int64
            lib.axon_reset()
        except Exception:
            pass
        res = run_bass_kernel_spmd(nc, in_maps, core_ids=list(range(NCORES)))

    out = np.empty((B, S, H, K), dtype=np.float32)
    for c in range(NCORES):
        b, g = c // 4, c % 4
        o = np.asarray(res.results[c]["out"]).astype(np.float32)  # [NH,65,S]
        ctx = o[:, 1:, :] / o[:, 0:1, :]   # normalize by denominators
        # [NH, K, S] -> [S, NH, K], plus bv
        out[b, :, NH * g:NH * (g + 1), :] = \
            ctx.transpose(2, 0, 1) + bv[NH * g:NH * (g + 1)][None]
    return out


# revision 22
# speedup vs baseline: 1.1131x; 1.1131x over previous
"""ALBERT attention (B=2, S=2048, D=1024, H=16, K=64) on 8 TRN2 NeuronCores.

Sharding: core c = (b, g) with b = c // 4 (batch), g = c % 4 (head group of 4
heads). Each core computes output[b, :, 4g:4g+4, :] — outputs are disjoint, so
no collectives are needed.

Host-side prep: keys with attention_mask == 0 are compacted away (they
contribute exactly 0), padded to a 128 multiple; only the LAST key tile
contains masked (padding) keys, so only its exp() needs the additive-mask
bias.  ALL inputs are host-packed into the EXACT SBUF tile layout
([128 partitions, free]) so every DMA moves >=4KB-contiguous per partition
(128 big descriptors per transfer instead of thousands of 256B ones).

Per-core pipeline (ScalarE exp is the roofline: 64 ACTs x ~1.11us):
  - DMAs ride the two HW-DGE queues (scalar, sync) for the critical prefix
    (wqk0 / xt parts 0-1 / xf quarter 0 / wv) and the gpsimd SW-DGE queue for
    late-needed data.  With packed layouts each dma_start occupies its engine
    for <1us, so the scalar queue is free for the exp stream.
  - projections, weight-stationary, bf16: QT [2-head 128, S] per pair and
    KT [128, T] per pair; V computed DIRECTLY in [t, hk] layout (xt chunks
    stationary, wv moving) — no PE transpose pass.
  - attention runs per (head-PAIR, f-quarter 512): the two heads' logits
    matmuls contract 64 rows each at partition offsets 0 / 64 so the PE runs
    them CONCURRENTLY into one lt [128, 1024] tile ([A | B]); a single exp
    ACT covers both; per-head contexts Cacc[65, 512] += [1|V]^T @ ET (row 0 =
    softmax denominators). PSUM: lt 2x2 + cacc 2x1 + proj 2 = 8 banks.
  - non-critical projections are chopped into SINGLE-MATMUL ops and INJECTED
    between attention iterations (program order = Tile priority), so the PE
    slack under the exp stream absorbs them instead of stalling it.
  - output ships UNNORMALIZED in bf16: out[h] = [65, S] (row 0 = denom,
    rows 1..65 = C^T). Host divides, transposes, and adds bv (exact since
    probs sum to 1).
"""

import ml_dtypes
import numpy as np

import concourse.bass as bass
import concourse.tile as tile
from concourse import bacc, mybir
from concourse.bass_utils import run_bass_kernel_spmd

F32 = mybir.dt.float32
BF16 = mybir.dt.bfloat16

B, S, D, H, K = 2, 2048, 1024, 16, 64
NH = 4            # heads per core
HK = NH * K       # 256
NCORES = 8
DC = D // 128     # 8 contraction chunks
NEG = -10000.0
FQ = 512          # f-quarter width
NQ = S // FQ      # 4 f-quarters
TP = 256          # xt part width == K-projection chunk width

# schedule knobs
N_WARM_BIG = 8    # 512-col warmup matmuls
N_WARM_SMALL = 4  # 128-col tail warmup matmuls (drain fast when work lands)
INJ = {(0, 0): 8, (0, 1): 4, (0, 2): 4, (0, 3): 2,
       (1, 0): 2, (1, 1): 2, (1, 2): 2, (1, 3): 0}


import os

_BISECT = os.environ.get("KBISECT", "")


def _part_widths(T):
    """xt part widths.  MUST be uniform: non-uniform xt tensor widths
    produce a NEFF the runtime rejects (INVALID_ARGUMENT) even though
    CoreSim validates the graph — bisected 2026-08-09."""
    w0 = 128 if "tp128" in _BISECT else 256
    ws, off = [], 0
    while off < T:
        w = min(w0, T - off)
        ws.append(w)
        off += w
    return ws


def build_nc(t_tiles: int):
    """Per-core Bass graph. t_tiles = number of 128-row key tiles after
    host-side compaction of masked-out keys."""
    T = t_tiles * 128
    pw = []                      # xt parts: (col offset, width)
    off = 0
    for w in _part_widths(T):
        pw.append((off, w))
        off += w
    NP = len(pw)

    nc = bacc.Bacc("TRN2", target_bir_lowering=False, debug=False,
                   num_devices=NCORES)

    # all inputs pre-packed host-side into [128, free] SBUF layout
    xf_d = [nc.dram_tensor(f"xf{j}", [128, DC * FQ], BF16,
                           kind="ExternalInput").ap() for j in range(NQ)]
    xt_d = [nc.dram_tensor(f"xt{i}", [128, DC * w], BF16,
                           kind="ExternalInput").ap()
            for i, (_, w) in enumerate(pw)]
    wqk_d = [nc.dram_tensor(f"wqk{hk}", [128, 2 * DC * 128], BF16,
                            kind="ExternalInput").ap() for hk in range(2)]
    wv_d = nc.dram_tensor("wv", [128, DC * HK], BF16,
                          kind="ExternalInput").ap()
    # bias columns: [bq0 bq1 bk0 bk1] (hk-tile halves of bq / bk)
    bias_d = nc.dram_tensor("bias", [128, 4], F32, kind="ExternalInput").ap()
    # additive key mask for the LAST key tile only
    mask_d = nc.dram_tensor("mask", [128, 1], F32, kind="ExternalInput").ap()
    # unnormalized: per head, row 0 = softmax denominators, rows 1..64 = C^T
    out_d = nc.dram_tensor("out", [NH, K + 1, S], BF16,
                           kind="ExternalOutput").ap()

    with tile.TileContext(nc) as tc:
        with (
            tc.sbuf_pool(name="const", bufs=1) as const_pool,
            tc.sbuf_pool(name="persist", bufs=1) as persist_pool,
            tc.psum_pool(name="proj", bufs=2) as proj_pool,
            tc.psum_pool(name="cacc", bufs=2) as cacc_pool,
            tc.sbuf_pool(name="et", bufs=t_tiles + 2) as et_pool,
            tc.sbuf_pool(name="ct", bufs=4) as ct_pool,
        ):
            bias_sb = const_pool.tile([128, 4], F32)
            mask_sb = const_pool.tile([128, 1], F32)
            warm_sb = const_pool.tile([128, 512], BF16)

            xf_sb = [persist_pool.tile([128, DC * FQ], BF16, name=f"xf{j}")
                     for j in range(NQ)]
            xt_sb = [persist_pool.tile([128, DC * w], BF16, name=f"xt{i}")
                     for i, (_, w) in enumerate(pw)]
            wqk_sb = [persist_pool.tile([128, 2 * DC * 128], BF16,
                                        name=f"wqk{hk}") for hk in range(2)]
            wv_sb = persist_pool.tile([128, DC * HK], BF16, name="wv")

            def wq_v(hk, d):     # [128, 128] stationary chunk
                return wqk_sb[hk][:, 128 * d:128 * (d + 1)]

            def wk_v(hk, d):
                return wqk_sb[hk][:, DC * 128 + 128 * d:
                                  DC * 128 + 128 * (d + 1)]

            def wv_v(d):         # [128, 256] moving chunk
                return wv_sb[:, HK * d:HK * (d + 1)]

            def xt_v(d, c0, w):  # [128, w] view of xt chunk d cols c0:c0+w
                for i, (o, wi) in enumerate(pw):
                    if o <= c0 and c0 + w <= o + wi:
                        s = wi * d + (c0 - o)
                        return xt_sb[i][:, s:s + w]
                raise AssertionError((c0, w))

            def xf_v(d, q):      # [128, 512] view of xf chunk d, quarter q
                return xf_sb[q][:, FQ * d:FQ * (d + 1)]

            qt_sb = [persist_pool.tile([128, S], BF16, name=f"qt{i}")
                     for i in range(2)]
            kt_sb = [persist_pool.tile([128, T], BF16, name=f"kt{i}")
                     for i in range(2)]
            # V with a leading ones column per head: [1|V_h0|1|V_h1|...]
            v_sb = [persist_pool.tile([128, NH * 65], BF16, name=f"v{i}")
                    for i in range(t_tiles)]
            nc.vector.memset(warm_sb[:], 0.0)
            for i in range(t_tiles):
                nc.vector.memset(
                    v_sb[i].rearrange("p (h c) -> p h c", c=65)[:, :, 0:1],
                    1.0)

            # ---------------- input DMAs ----------------
            # The two HW-DGE queues (scalar + sync) carry the critical
            # prefix: everything attention(0,0) touches.  Late-needed data
            # (xf quarters 1-3, pair-1 weights) rides the gpsimd SW-DGE
            # queue GATED behind the Q00 drain so it cannot steal HBM
            # bandwidth from the critical prefix.  Packed layouts make each
            # dma_start issue in <1us of engine time.
            if "v1dma" in _BISECT:
                nc.scalar.dma_start(bias_sb[:], bias_d[:])
                nc.scalar.dma_start(mask_sb[:], mask_d[:])
                nc.scalar.dma_start(wqk_sb[0][:], wqk_d[0][:])
                nc.scalar.dma_start(xt_sb[0][:], xt_d[0][:])
                nc.scalar.dma_start(xt_sb[1][:], xt_d[1][:])
                nc.sync.dma_start(xf_sb[0][:], xf_d[0][:])
                nc.sync.dma_start(wv_sb[:], wv_d[:])
                nc.sync.dma_start(wqk_sb[1][:], wqk_d[1][:])
                nc.sync.dma_start(xf_sb[1][:], xf_d[1][:])
                for i in range(2, NP):
                    nc.gpsimd.dma_start(xt_sb[i][:], xt_d[i][:])
                nc.gpsimd.dma_start(xf_sb[2][:], xf_d[2][:])
                nc.gpsimd.dma_start(xf_sb[3][:], xf_d[3][:])
            else:
                # critical prefix on the two HW-DGE queues; xf0 split in
                # half across both so Q00 unblocks as early as possible
                HF = (4 * FQ) if "split" in _BISECT else 0
                nc.scalar.dma_start(bias_sb[:], bias_d[:])
                nc.scalar.dma_start(mask_sb[:], mask_d[:])
                nc.scalar.dma_start(wqk_sb[0][:], wqk_d[0][:])
                if HF:
                    nc.scalar.dma_start(xf_sb[0][:, HF:], xf_d[0][:, HF:])
                nc.scalar.dma_start(wv_sb[:], wv_d[:])
                for i in range(NP):
                    if i == 2:
                        nc.sync.dma_start(xf_sb[0][:, 0:HF] if HF
                                          else xf_sb[0][:],
                                          xf_d[0][:, 0:HF] if HF
                                          else xf_d[0][:])
                    nc.sync.dma_start(xt_sb[i][:], xt_d[i][:])
                if NP <= 2:
                    nc.sync.dma_start(xf_sb[0][:, 0:HF] if HF else xf_sb[0][:],
                                      xf_d[0][:, 0:HF] if HF else xf_d[0][:])
                # late-needed inputs ride the gpsimd SW-DGE queue, gated
                # behind the Q00 drain (see schedule section) so they can't
                # steal HBM bandwidth from the critical prefix

            # -------- projection pieces as lists of small ops --------
            def ops_q(hk, q):
                st = {}
                ops = [lambda: st.__setitem__(
                    "ps", proj_pool.tile([128, FQ], F32, tag="pp",
                                         name=f"qp{hk}_{q}"))]
                for d in range(DC):
                    ops.append(lambda d=d: nc.tensor.matmul(
                        st["ps"][:], wq_v(hk, d), xf_v(d, q),
                        start=(d == 0), stop=(d == DC - 1)))
                ops.append(lambda: nc.vector.tensor_scalar_add(
                    qt_sb[hk][:, FQ * q:FQ * (q + 1)], st["ps"][:],
                    bias_sb[:, hk:hk + 1]))
                return ops

            def ops_k(hk, ci):
                c0, w = pw[ci]
                st = {}
                ops = [lambda: st.__setitem__(
                    "ps", proj_pool.tile([128, w], F32, tag="pp",
                                         name=f"kp{hk}_{ci}"))]
                for d in range(DC):
                    ops.append(lambda d=d: nc.tensor.matmul(
                        st["ps"][:], wk_v(hk, d), xt_v(d, c0, w),
                        start=(d == 0), stop=(d == DC - 1)))
                ops.append(lambda: nc.vector.tensor_scalar_add(
                    kt_sb[hk][:, c0:c0 + w], st["ps"][:],
                    bias_sb[:, 2 + hk:3 + hk]))
                return ops

            def ops_v(t):
                """bv is added on the host (exact: probs sum to 1)."""
                st = {}
                ops = [lambda: st.__setitem__(
                    "ps", proj_pool.tile([128, HK], F32, tag="pp",
                                         name=f"vp{t}"))]
                for d in range(DC):
                    ops.append(lambda d=d: nc.tensor.matmul(
                        st["ps"][:], xt_v(d, 128 * t, 128), wv_v(d),
                        start=(d == 0), stop=(d == DC - 1)))
                ops.append(lambda: nc.vector.tensor_copy(
                    v_sb[t].rearrange("p (h c) -> p h c", c=65)[:, :, 1:65],
                    st["ps"].rearrange("p (h c) -> p h c", c=64)[:, :, :]))
                return ops

            # pending background ops: (deadline_slot, op).  deadline_slot is
            # the absolute attention slot (call_index * t_tiles + tile)
            # BEFORE whose logits matmul the op must have executed; the list
            # is kept in non-decreasing deadline order so FIFO execution is
            # both the pacing order and the correctness order.
            bg = []

            def bg_add(deadline, piece):
                assert not bg or bg[-1][0] <= deadline
                bg.extend((deadline, op) for op in piece)

            def bg_pop(n):
                for _ in range(min(n, len(bg))):
                    bg.pop(0)[1]()

            def bg_run_due(slot):
                while bg and bg[0][0] <= slot:
                    bg.pop(0)[1]()

            def bg_flush():
                bg_pop(len(bg))

            def attention(hk, fq, lt_pool, base, inject=3):
                """Both heads of pair hk on f-quarter fq: concurrent
                row-tiled logits -> one exp ACT -> per-head contexts."""
                hA, hB = 2 * hk, 2 * hk + 1
                c0 = FQ * fq
                caccs = [cacc_pool.tile([K + 1, FQ], F32, tag="cacc",
                                        name=f"cacc{hk}_{fq}_{i}")
                         for i in range(2)]
                for t in range(t_tiles):
                    bg_run_due(base + t)
                    lt = lt_pool.tile([128, 2 * FQ], F32, tag="lt",
                                      name=f"lt{hk}_{fq}_{t}")
                    for i, zo in ((0, 0), (1, 64)):
                        nc.tensor.matmul(
                            lt[:, FQ * i:FQ * (i + 1)],
                            kt_sb[hk][zo:zo + 64, 128 * t:128 * (t + 1)],
                            qt_sb[hk][zo:zo + 64, c0:c0 + FQ],
                            start=True, stop=True)
                    et = et_pool.tile([128, 2 * FQ], BF16, tag="et",
                                      name=f"et{hk}_{fq}_{t}")
                    nc.scalar.activation(
                        et[:], lt[:], mybir.ActivationFunctionType.Exp,
                        bias=(mask_sb[:, 0:1] if t == t_tiles - 1 else 0.0),
                        scale=0.125)
                    bg_pop(inject)
                    for i, h in ((0, hA), (1, hB)):
                        nc.tensor.matmul(
                            caccs[i][:],
                            v_sb[t][:, 65 * h:65 * (h + 1)],
                            et[:, FQ * i:FQ * (i + 1)],
                            start=(t == 0), stop=(t == t_tiles - 1),
                            skip_group_check=True)
                for i, h in ((0, hA), (1, hB)):
                    ct = ct_pool.tile([K + 1, FQ], BF16, tag="ct",
                                      name=f"ct{hk}_{fq}_{i}")
                    nc.vector.tensor_copy(ct[:], caccs[i][:])
                    nc.sync.dma_start(out_d[h][:, c0:c0 + FQ], ct[:])

            # ---------------- schedule ----------------
            # PE warmup: keep the HAM clock-gate hot until real work lands;
            # small tail matmuls drain fast once inputs arrive.
            with tc.psum_pool(name="warm", bufs=1) as warm_pool:
                wps = warm_pool.tile([128, 512], F32, tag="wp", name="warm")
                for _ in range(N_WARM_BIG):
                    nc.tensor.matmul(wps[:], warm_sb[:, 0:128], warm_sb[:],
                                     start=True, stop=True)
                for _ in range(N_WARM_SMALL):
                    nc.tensor.matmul(wps[:, 0:128], warm_sb[:, 0:128],
                                     warm_sb[:, 0:128], start=True, stop=True)

            # upfront: what attention(0,0) tiles 0/1 need, ordered to match
            # DMA arrival (xt0, wv, xt1 land before xf0's second half)
            up_parts = min(3 if "up3" in _BISECT else 2, NP)
            pieces = []
            for ci in range(up_parts):
                pieces.append(ops_k(0, ci))
                for t in range(pw[ci][0] // 128,
                               (pw[ci][0] + pw[ci][1]) // 128):
                    pieces.append(ops_v(t))
            pieces.append(ops_q(0, 0))
            for piece in pieces:
                for op in piece:
                    op()

            # gated SW-DGE DMAs: the gpsimd queue only starts pulling HBM
            # bandwidth once the Q00 drain has run (i.e. the critical
            # prefix has landed).  Not under v1dma (those DMAs were issued
            # upfront there).
            if "v1dma" not in _BISECT:
                # TRUE-dependency gate (program order alone is NOT a
                # schedule: the Tile scheduler hoists ready instructions).
                # Each gated DMA's dst tile first gets a 1-column vector
                # write that READS qt (RAW on the Q00 drain), so the DMA
                # (WAW on that column) cannot start before the critical
                # prefix has been consumed.
                gated = [(xf_sb[1], xf_d[1]), (xf_sb[2], xf_d[2]),
                         (wqk_sb[1], wqk_d[1]), (xf_sb[3], xf_d[3])]
                if "nogate" not in _BISECT:
                    for dst, _ in gated:
                        nc.vector.tensor_copy(dst[:, 0:1], qt_sb[0][:, 0:1])
                for dst, srcd in gated:
                    nc.gpsimd.dma_start(dst[:], srcd[:])

            # background work with absolute slot deadlines
            TT = t_tiles
            for ci in range(up_parts, NP):
                t0c = pw[ci][0] // 128
                bg_add(t0c, ops_k(0, ci))
                for t in range(t0c, (pw[ci][0] + pw[ci][1]) // 128):
                    bg_add(t, ops_v(t))
            bg_add(1 * TT, ops_q(0, 1))
            bg_add(2 * TT, ops_q(0, 2))
            bg_add(3 * TT, ops_q(0, 3))
            bg_add(4 * TT, ops_q(1, 0))
            for ci in range(NP):
                bg_add(4 * TT + pw[ci][0] // 128, ops_k(1, ci))
            bg_add(5 * TT, ops_q(1, 1))
            bg_add(6 * TT, ops_q(1, 2))
            bg_add(7 * TT, ops_q(1, 3))

            with tc.psum_pool(name="lt", bufs=2) as lt_pool:
                for ci, (hk, fq) in enumerate(
                        [(h, q) for h in range(2) for q in range(NQ)]):
                    attention(hk, fq, lt_pool, base=ci * TT,
                              inject=INJ[(hk, fq)])
                bg_flush()

    nc.compile()
    return nc


_NC_CACHE = {}


def _get_nc(t_tiles: int):
    if t_tiles not in _NC_CACHE:
        _NC_CACHE[t_tiles] = build_nc(t_tiles)
    return _NC_CACHE[t_tiles]


def _pack_cols(a):
    """[D, W] (d*128+p, w) -> [128, DC*W] SBUF layout [p, d*W+w]."""
    Wd = a.shape[1]
    return np.ascontiguousarray(
        a.reshape(DC, 128, Wd).transpose(1, 0, 2).reshape(128, DC * Wd))


def kernel(from_tensor, to_tensor, attention_mask, Wq, bq, Wk, bk, Wv, bv):
    from_tensor = np.asarray(from_tensor, dtype=np.float32)
    to_tensor = np.asarray(to_tensor, dtype=np.float32)
    attention_mask = np.asarray(attention_mask)
    Wq = np.asarray(Wq, dtype=np.float32)
    Wk = np.asarray(Wk, dtype=np.float32)
    Wv = np.asarray(Wv, dtype=np.float32)
    bq = np.asarray(bq, dtype=np.float32)
    bk = np.asarray(bk, dtype=np.float32)
    bv = np.asarray(bv, dtype=np.float32)

    # compact away masked-out keys (they contribute exactly 0 to the
    # context); pad to a 128 multiple and re-mask the padding tail.
    mask_np = attention_mask.astype(np.int32)
    idxs = [np.nonzero(mask_np[b])[0] for b in range(B)]
    t_eff = max(1, max(len(ix) for ix in idxs))
    T_pad = min(S, ((t_eff + 127) // 128) * 128)
    t_tiles = T_pad // 128
    nc = _get_nc(t_tiles)

    pw = []
    off = 0
    for w in _part_widths(T_pad):
        pw.append((off, w))
        off += w

    # per-batch packed activations (shared by the 4 cores of each batch)
    xf_pack = []   # [b][j] -> [128, DC*FQ] bf16
    xt_pack = []   # [b][i] -> [128, DC*w] bf16
    maskadd = np.full((B, T_pad), NEG, dtype=np.float32)
    for b in range(B):
        ix = idxs[b]
        xfT = np.ascontiguousarray(from_tensor[b].T).astype(ml_dtypes.bfloat16)
        xf_pack.append([_pack_cols(xfT[:, FQ * j:FQ * (j + 1)])
                        for j in range(NQ)])
        xt_c = np.zeros((D, T_pad), dtype=np.float32)
        xt_c[:, :len(ix)] = to_tensor[b].T[:, ix]
        xt_b = xt_c.astype(ml_dtypes.bfloat16)
        xt_pack.append([_pack_cols(xt_b[:, o:o + w]) for o, w in pw])
        maskadd[b, :len(ix)] = 0.0

    in_maps = []
    for c in range(NCORES):
        b, g = c // 4, c % 4
        hs = slice(NH * g, NH * (g + 1))
        wq_sl = Wq[:, hs, :].reshape(D, HK)
        wk_sl = Wk[:, hs, :].reshape(D, HK)
        wv_sl = Wv[:, hs, :].reshape(D, HK)
        im = {
            "wv": _pack_cols(wv_sl).astype(ml_dtypes.bfloat16),
            "bias": np.ascontiguousarray(np.stack([
                bq[hs].reshape(HK)[:128], bq[hs].reshape(HK)[128:],
                bk[hs].reshape(HK)[:128], bk[hs].reshape(HK)[128:],
            ], axis=1)),
            "mask": np.ascontiguousarray(
                maskadd[b][(t_tiles - 1) * 128:].reshape(128, 1)),
        }
        for hk in range(2):
            sl = slice(128 * hk, 128 * (hk + 1))
            im[f"wqk{hk}"] = np.concatenate(
                [_pack_cols(wq_sl[:, sl]), _pack_cols(wk_sl[:, sl])],
                axis=1).astype(ml_dtypes.bfloat16)
        for j in range(NQ):
            im[f"xf{j}"] = xf_pack[b][j]
        for i in range(len(pw)):
            im[f"xt{i}"] = xt_pack[b][i]
        in_maps.append(im)

    global _LAST_IN_MAPS, _LAST_T_TILES
    _LAST_IN_MAPS = in_maps
    _LAST_T_TILES = t_tiles
    try:
        res = run_bass_kernel_spmd(nc, in_maps, core_ids=list(range(NCORES)))
    except Exception:
        # the axon terminal occasionally reports the device unrecoverable;
        # a reset + retry clears it
        try:
            import ctypes

            lib = ctypes.CDLL("/opt/axon/libaxon_pjrt.so")
            lib.axon_reset.restype = ctypes.c_int64
            lib.axon_reset()
        except Exception:
            pass
        res = run_bass_kernel_spmd(nc, in_maps, core_ids=list(range(NCORES)))

    out = np.empty((B, S, H, K), dtype=np.float32)
    for c in range(NCORES):
        b, g = c // 4, c % 4
        o = np.asarray(res.results[c]["out"]).astype(np.float32)  # [NH,65,S]
        ctx = o[:, 1:, :] / o[:, 0:1, :]   # normalize by denominators
        # [NH, K, S] -> [S, NH, K], plus bv
        out[b, :, NH * g:NH * (g + 1), :] = \
            ctx.transpose(2, 0, 1) + bv[NH * g:NH * (g + 1)][None]
    return out
